# revision 1
# baseline (speedup 1.0000x reference)
# BinsCombinerLayer TRN2 kernel — fp8(e3m4) TensorEngine, DMA-roofline.
#
#   out[b] = (1/16) * sum_{n,s} inputs[b,n,s] * centroids[n,s]
#
# The op is a pure matrix-vector product streamed once from HBM, so the
# kernel is bandwidth-bound and the lever is bytes/element.  Each bin's
# 128 probabilities sum to exactly 1, so for any offsets
#   out[b] = Koff + (1/16) <x_b - 1/128, c - mean(c_n)>,
#   Koff = sum_n mean(c_n) / 16   (added on host during unshard).
# The centered residual d = x - 1/128 is ~8x smaller than x, which makes
# e3m4 (4-bit mantissa) quantization of d*1024 accurate enough: scale-rel
# error vs the f32 reference is 1.371e-2 (gate 2e-2), verified bit-exact
# between the numpy model, CoreSim, and hardware (e3m4 products are exact
# on the PE at FP22; accumulation is f32).
#
# Per core (4096 examples = 8 MB fp8), feature-major layout: per block of
# 512 examples, 16 chunk tiles [128 feats, 512 ex]; c is the stationary
# operand ([128,1] per chunk) and 16 accumulating (K=128, M=1, N=512)
# matmuls produce the block's PSUM row.  The four blocks of a DMA group
# run on the four 128x32 PE column tiles (tile_position=(0,32j)) — each
# column tile has its own XBUS stream, so 4-tile ingest (>2 Telem/s
# measured) far exceeds the ~0.95 TB/s DMA roofline that bounds the
# kernel at ~9-12 us/pass.  DMAs carry 8 chunks x 4 blocks (1 MB, 4 KB
# contiguous per partition) so matmul waves start while the rest of the
# group streams; ScalarE drains PSUM with a fused *1/(16*sd*sc); outputs
# DMA out per 4-block group.
import numpy as np
import ml_dtypes

import concourse.bacc as bacc
import concourse.mybir as mybir
import concourse.tile as tile
from concourse.bass_utils import run_bass_kernel_spmd

N_CORES = 8
B, NUM_BINS, BIN_SIZE = 32768, 16, 128
D = NUM_BINS * BIN_SIZE
P = 128
BC = B // N_CORES
NBLK = 512
BLOCKS = BC // NBLK          # 8
CHUNKS = D // P              # 16
SD = 1024.0
SC = 4.0
ALPHA = 1.0 / (NUM_BINS * SD * SC)
F32 = mybir.dt.float32
F8 = mybir.dt.float8e3
E3M4 = ml_dtypes.float8_e3m4

_CACHED = {}


def _build_program(repeat=1, blocks_per_dma=4, bufs=10, drain="act", out_per_group=True, chunk_step=1, cs=8):
    nc = bacc.Bacc("TRN2", target_bir_lowering=False, debug=False)
    x = nc.dram_tensor(
        "x", [P, BLOCKS, CHUNKS, NBLK], F8, kind="ExternalInput"
    ).ap()
    cb = nc.dram_tensor("cb", [P, CHUNKS], F8, kind="ExternalInput").ap()
    out = nc.dram_tensor(
        "out", [4, (BLOCKS // 4) * NBLK], F32, kind="ExternalOutput"
    ).ap()

    with tile.TileContext(nc) as tc:
        with (
            tc.tile_pool(name="xin", bufs=bufs) as xpool,
            tc.tile_pool(name="misc", bufs=1) as misc,
            tc.tile_pool(name="cl", bufs=2) as clpool,
            tc.tile_pool(name="ps", bufs=1, space="PSUM") as pspool,
        ):
            ct = misc.tile([P, CHUNKS], F8)
            nc.sync.dma_start(out=ct[:], in_=cb[:])
            psum = [
                pspool.tile([P, NBLK], F32, name=f"psum{b}")
                for b in range(BLOCKS)
            ]

            for _ in range(repeat):
                collect = clpool.tile([P, (BLOCKS // 4) * NBLK], F32, tag="cl")
                for d in range(BLOCKS // blocks_per_dma):
                    b0 = d * blocks_per_dma
                    # cs chunks per DMA: finer waves let matmuls start while
                    # the rest of the group's data still streams (shorter
                    # ramp and post-DMA tail); the DRAM slice stays
                    # descriptor-efficient (cs*512 B contiguous per block
                    # per partition).
                    for h in range(CHUNKS // cs):
                        xt = xpool.tile(
                            [P, blocks_per_dma, cs, NBLK], F8, tag="xt"
                        )
                        nc.sync.dma_start(
                            out=xt[:],
                            in_=x[
                                :,
                                b0 : b0 + blocks_per_dma,
                                h * cs : (h + 1) * cs,
                            ],
                        )
                        for qq in range(0, cs, chunk_step):
                            q = h * cs + qq
                            for i in range(blocks_per_dma):
                                b = b0 + i
                                j = b % 4
                                nc.tensor.matmul(
                                    psum[b][32 * j : 32 * j + 1, :],
                                    ct[:, q : q + 1],
                                    xt[:, i, qq, :],
                                    start=(q == 0),
                                    stop=(q >= CHUNKS - chunk_step),
                                    tile_position=(0, 32 * j),
                                )
                    for i in range(blocks_per_dma):
                        b = b0 + i
                        j = b % 4
                        dst = collect[
                            32 * j : 32 * j + 1,
                            (b // 4) * NBLK : (b // 4 + 1) * NBLK,
                        ]
                        srcp = psum[b][32 * j : 32 * j + 1, :]
                        if drain == "act":
                            nc.scalar.activation(
                                dst, srcp,
                                mybir.ActivationFunctionType.Copy,
                                scale=ALPHA,
                            )
                        else:
                            nc.vector.tensor_scalar_mul(dst, srcp, ALPHA)
                        if out_per_group and (b % 4) == 3:
                            g4 = b // 4
                            nc.sync.dma_start(
                                out=out[:, g4 * NBLK : (g4 + 1) * NBLK],
                                in_=collect[0 : P : 32, g4 * NBLK : (g4 + 1) * NBLK],
                            )
                if not out_per_group:
                    nc.sync.dma_start(out=out[:], in_=collect[0 : P : 32, :])

    nc.compile()
    return nc


def _get_program():
    if "main" not in _CACHED:
        _CACHED["main"] = _build_program()
    return _CACHED["main"]


def prepare(inputs, centroids):
    x = np.asarray(inputs, dtype=np.float32).reshape(
        N_CORES, BLOCKS, NBLK, CHUNKS, P
    )
    d = x - np.float32(1.0 / 128.0)
    d *= np.float32(SD)
    np.clip(d, -15.5, 15.5, out=d)
    dq = d.astype(E3M4)
    dq = np.ascontiguousarray(dq.transpose(0, 4, 1, 3, 2))

    c = np.asarray(centroids, dtype=np.float32).reshape(NUM_BINS, BIN_SIZE)
    cbar = c.mean(axis=1)
    dc = (c - cbar[:, None]) * np.float32(SC)
    cq = np.ascontiguousarray(np.clip(dc, -15.5, 15.5).astype(E3M4).T)
    koff = np.float32(cbar.sum() / NUM_BINS)
    return [{"x": dq[i], "cb": cq} for i in range(N_CORES)], koff


def unpack(results, koff):
    outs = [
        r["out"]
        .reshape(4, BLOCKS // 4, NBLK)
        .transpose(1, 0, 2)
        .reshape(BC)
        for r in results
    ]
    return (np.concatenate(outs) + koff).astype(np.float32, copy=False)


def run(inputs, centroids, **spmd_kwargs):
    nc = _get_program()
    in_maps, koff = prepare(inputs, centroids)
    full, res = None, None
    for attempt in range(3):
        try:
            res = run_bass_kernel_spmd(
                nc, in_maps, list(range(N_CORES)), **spmd_kwargs
            )
        except Exception:
            # transient axon/NRT wedges (mesh desync, NRT_EXEC_UNIT_...)
            # recover on retry
            if attempt == 2:
                raise
            continue
        full = unpack(res.results, koff)
        # a desynced device can return silently-corrupt buffers; finite
        # inputs must produce finite outputs, so treat NaN/Inf as a
        # failed execution and retry
        if np.isfinite(full).all():
            return full, res
    return full, res


def kernel(inputs, centroids):
    full, _ = run(inputs, centroids)
    return full

